# revision 70
# baseline (speedup 1.0000x reference)
"""GCN cascade layer (3 parallel GCNConv + 1 linear head) on 8 Trainium2 cores.

Math (per edge set i):
    deg[c] = sum_{e: col=c} w[e];  dinv = deg>0 ? rsqrt(deg) : 0
    out[c] = relu( sum_e dinv[row]*w*dinv[col] * (x[row] @ W_i.T) + b_i )

Two host-side folds make the device program minimal:
 1. GCN norm folded into per-edge weights norm[e] = dinv[row]*w*dinv[col].
 2. The conv is linear, so aggregation happens BEFORE the weight matmul:
        t_i[c]  = sum_{e->c} norm_i[e] * x[row[e]]   (gather + scatter of RAW x)
        out_i[c] = relu( t_i[c] @ W_i.T + b_i )
    No dense pre-pass over all nodes, no intermediate h in DRAM.

Device per (set, output-tile): SWDGE dma_gather of x rows into lane chunks,
one-hot-weighted matmuls ACCUMULATING THE TRANSPOSE t^T[fin, c] in PSUM
(lhsT = gathered x chunk, rhs = one-hot weights — so no transpose is ever
needed), cast to fp16, then one [P,P] matmul out^T = W_i @ t^T whose relu
and per-partition bias fuse into a single Activation op; outputs are
written transposed [fout, node] (big DMA descriptors) and transposed back
on the host. Output nodes are sharded over 8 cores in contiguous 128-col
tiles; x is replicated; edges are bucketed by (output tile, lo/hi source
half) on the host (gather indices are int16, so sources split in halves).
Engine assignment: Pool = gathers only; DVE = one-hot builds (the pacing
resource, ~94ns each); Act = casts + fused bias/relu + output DMA queue;
SP = input DMA queue; PE = all matmuls. The W-matmul of tile j issues
after tile j+1's scatter matmuls (software pipelining) and all tile pools
have single-engine writers to avoid cross-engine WAW event semaphores.
"""

import sys

sys.path.insert(0, "/opt/trn_rl_repo")

import math
from dataclasses import dataclass, field

import numpy as np

import concourse.bass as bass
import concourse.bacc as bacc
import concourse.mybir as mybir
from concourse import tile

P = 128          # partitions / feature dim
CORES = 8
DG = 7           # tiles per scatter group

f16 = mybir.dt.float16
f32 = mybir.dt.float32
i16 = mybir.dt.int16

LAST_RESULTS = None
TRACE = False


@dataclass
class Cfg:
    N: int
    E: int
    A: int
    TPC: int          # node tiles per core
    NT: int           # total node tiles (CORES*TPC)
    N2: int           # padded node count (NT*P)
    OWN: int          # cols/rows owned per core (TPC*P)
    LO_T: int
    LO_ROWS: int
    HI_T: int
    HI_ROWS: int
    K_lo: int = 0
    K_hi: int = 0
    K_u: int = 0
    n_groups: int = 0
    group_tiles: list = field(default_factory=list)
    node_of_slot: object = None


def _make_cfg(N, E, A):
    TPC = math.ceil(N / (CORES * P))
    NT = CORES * TPC
    N2 = NT * P
    LO_T = (NT + 1) // 2
    LO_ROWS = LO_T * P
    HI_T = NT - LO_T
    HI_ROWS = HI_T * P
    assert LO_ROWS < 32768 and HI_ROWS < 32768, "int16 gather index overflow"
    cfg = Cfg(N=N, E=E, A=A, TPC=TPC, NT=NT, N2=N2, OWN=TPC * P,
              LO_T=LO_T, LO_ROWS=LO_ROWS, HI_T=HI_T, HI_ROWS=HI_ROWS)
    g = min(DG, TPC)
    cfg.n_groups = math.ceil(TPC / g)
    cfg.group_tiles = [min(g, TPC - i * g) for i in range(cfg.n_groups)]
    return cfg


def _prep(cfg, x, edge_index, edge_attr, lin_w, lin_b, conv_w, conv_b):
    """Host-side sharding/layout prep. Returns per-core input dict list."""
    A, N, E = cfg.A, cfg.N, cfg.E
    TPC, NT = cfg.TPC, cfg.NT

    r_all = edge_index[:, 0, :].astype(np.int64)   # [A,E]
    c_all = edge_index[:, 1, :].astype(np.int64)
    w_all = edge_attr.astype(np.float64)

    # --- fold gcn norm into per-edge weights (host) ---
    norm_all = np.empty((A, E), np.float32)
    for i in range(A):
        deg = np.zeros(cfg.N2, np.float64)
        np.add.at(deg, c_all[i], w_all[i])
        dinv = np.where(deg > 0, 1.0 / np.sqrt(np.maximum(deg, 1e-300)), 0.0)
        norm_all[i] = (dinv[r_all[i]] * w_all[i] * dinv[c_all[i]]).astype(np.float32)

    # --- balanced node->tile permutation (3-dim greedy vector packing of
    # per-set tile totals, capped at 2*1024-2, + one 2-opt swap round) and a
    # SOURCE-side balanced lo/hi split (greedy signed + sideways repair).
    # Goal: every (set, tile, half) bucket <= 1024 edges -> K = 8+8. K is
    # always derived from actual bucket maxima, so this degrades gracefully.
    NT_ = NT
    CAPT = 2 * 1024 - 2
    dvec = np.zeros((cfg.N2, A), np.int32)
    for i in range(A):
        np.add.at(dvec[:, i], c_all[i], 1)
    order_n = np.argsort(-dvec.sum(1), kind="stable")
    loads = np.zeros((NT_, A), np.int32)
    counts = np.zeros(NT_, np.int32)
    tile_of_node = np.empty(cfg.N2, np.int32)
    for n in order_n:
        cand = np.flatnonzero(counts < P)
        post = loads[cand] + dvec[n]
        sc = np.maximum(post - CAPT, 0).sum(axis=1) * 100000 + post.max(axis=1)
        b = cand[np.argmin(sc)]
        tile_of_node[n] = b
        loads[b] += dvec[n]
        counts[b] += 1
    nodes_in = [np.flatnonzero(tile_of_node == t) for t in range(NT_)]
    for _ in range(8):
        bad = np.flatnonzero((loads > CAPT).any(1))
        if bad.size == 0:
            break
        for tb in bad:
            if not (loads[tb] > CAPT).any():
                continue
            done = False
            for tg_ in np.argsort(loads.max(1)):
                if done or tg_ == tb:
                    continue
                ng_list = nodes_in[tg_]
                for nb in nodes_in[tb]:
                    d_b = dvec[nb]
                    ok = ((loads[tb] - d_b + dvec[ng_list] <= CAPT).all(1) &
                          (loads[tg_] + d_b - dvec[ng_list] <= CAPT).all(1))
                    if ok.any():
                        ng = ng_list[int(np.flatnonzero(ok)[0])]
                        loads[tb] += dvec[ng] - d_b
                        loads[tg_] += d_b - dvec[ng]
                        tile_of_node[nb] = tg_
                        tile_of_node[ng] = tb
                        nodes_in[tb] = np.append(
                            np.setdiff1d(nodes_in[tb], [nb]), ng)
                        nodes_in[tg_] = np.append(
                            np.setdiff1d(nodes_in[tg_], [ng]), nb)
                        done = True
                        break
    lane_of_node = np.empty(cfg.N2, np.int32)
    for t in range(NT_):
        mem = np.flatnonzero(tile_of_node == t)
        lane_of_node[mem] = np.arange(mem.size)
    node_of_slot = np.empty(cfg.N2, np.int64)
    node_of_slot[tile_of_node.astype(np.int64) * P + lane_of_node] = \
        np.arange(cfg.N2)
    cfg.node_of_slot = node_of_slot

    # source-side split: balance each (set, tile) bucket's lo-count to half
    bid_all = np.concatenate([i * NT_ + tile_of_node[c_all[i]]
                              for i in range(A)]).astype(np.int64)
    src_all = np.concatenate([r_all[i] for i in range(A)])
    so_ = np.argsort(src_all, kind="stable")
    bid_s_ = bid_all[so_]
    st_ = np.searchsorted(src_all[so_], np.arange(cfg.N2))
    en_ = np.searchsorted(src_all[so_], np.arange(cfg.N2) + 1)
    odeg = en_ - st_
    totals_b = np.bincount(bid_all, minlength=A * NT_)
    rng = np.random.default_rng(2)
    locnt = np.zeros(A * NT_, np.int32)
    defc = np.zeros(A * NT_, np.int32)
    side = np.zeros(cfg.N2, np.bool_)
    nlo = 0
    for n in np.argsort(-odeg + rng.uniform(0, 0.5, cfg.N2), kind="stable"):
        bs = bid_s_[st_[n]:en_[n]]
        sdef = defc[bs].sum()
        lo = sdef < 0 or (sdef == 0 and rng.random() < 0.5)
        np.add.at(defc, bs, 1 if lo else -1)
        if lo:
            np.add.at(locnt, bs, 1)
            nlo += 1
        side[n] = lo
    lo_max_sz = 32767
    hi_min = cfg.N2 - lo_max_sz
    import time as _time
    _t0 = _time.time()
    for sweep in range(60):
        over = (np.maximum(locnt - 1024, 0) +
                np.maximum(totals_b - locnt - 1024, 0)).sum()
        if over == 0 or _time.time() - _t0 > 40:
            break
        sideways = sweep % 3 == 2
        for n in rng.permutation(cfg.N2):
            if odeg[n] == 0:
                continue
            bs = bid_s_[st_[n]:en_[n]]
            cur = (np.maximum(locnt[bs] - 1024, 0) +
                   np.maximum(totals_b[bs] - locnt[bs] - 1024, 0)).sum()
            if cur == 0:
                continue
            if side[n]:
                new = (np.maximum(locnt[bs] - 1 - 1024, 0) +
                       np.maximum(totals_b[bs] - locnt[bs] + 1 - 1024, 0)).sum()
                ok = new < cur or (sideways and new == cur
                                   and rng.random() < 0.3)
                if ok and (nlo - 1) >= hi_min:
                    np.add.at(locnt, bs, -1)
                    side[n] = False
                    nlo -= 1
            else:
                new = (np.maximum(locnt[bs] + 1 - 1024, 0) +
                       np.maximum(totals_b[bs] - locnt[bs] - 1 - 1024, 0)).sum()
                ok = new < cur or (sideways and new == cur
                                   and rng.random() < 0.3)
                if ok and (nlo + 1) <= lo_max_sz:
                    np.add.at(locnt, bs, 1)
                    side[n] = True
                    nlo += 1
    lo_nodes = np.flatnonzero(side)
    hi_nodes = np.flatnonzero(~side)
    assert 0 < lo_nodes.size < 32768 and 0 < hi_nodes.size < 32768
    idx_in_half = np.empty(cfg.N2, np.int64)
    idx_in_half[lo_nodes] = np.arange(lo_nodes.size)
    idx_in_half[hi_nodes] = np.arange(hi_nodes.size)
    cfg.LO_ROWS = int(lo_nodes.size)
    cfg.HI_ROWS = int(hi_nodes.size)
    cfg.LO_T = 1
    cfg.HI_T = 1

    # --- edge bucketing: per (set, out-tile, lo/hi-half), rank within ---
    K_lo = K_hi = 0
    per_set = []
    for i in range(A):
        c, r, w = c_all[i], r_all[i], norm_all[i]
        tile_of = tile_of_node[c].astype(np.int64)
        lane_c = lane_of_node[c].astype(np.int64)
        is_hi = (~side[r]).astype(np.int64)
        order = np.lexsort((is_hi, tile_of))
        c_s, r_s, w_s, t_s, hi_s = lane_c[order], r[order], w[order], tile_of[order], is_hi[order]
        seg_key = t_s * 2 + hi_s
        seg_change = np.empty(E, np.bool_)
        seg_change[0] = True
        seg_change[1:] = seg_key[1:] != seg_key[:-1]
        starts = np.zeros(E, np.int64)
        idx = np.flatnonzero(seg_change)
        starts[idx] = idx
        starts = np.maximum.accumulate(starts)
        rank = np.arange(E) - starts
        n_lo = np.bincount(t_s[hi_s == 0], minlength=NT)
        n_hi = np.bincount(t_s[hi_s == 1], minlength=NT)
        K_lo = max(K_lo, int(math.ceil(n_lo.max() / P)))
        K_hi = max(K_hi, int(math.ceil(n_hi.max() / P)))
        per_set.append((c_s, r_s, w_s, t_s, hi_s, rank))

    K_lo = max(K_lo, 1)
    K_hi = max(K_hi, 1) if cfg.HI_T > 0 else 0
    cfg.K_lo, cfg.K_hi, cfg.K_u = K_lo, K_hi, K_lo + K_hi

    CH = TPC * cfg.K_u
    colloc = np.zeros((CORES, A, P, CH), np.float32)
    wchunk = np.zeros((CORES, A, P, CH), np.float32)
    gidx_lo = np.zeros((CORES, A, 16, TPC * K_lo * 8), np.int16)
    gidx_hi = np.zeros((CORES, A, 16, max(TPC * K_hi * 8, 1)), np.int16)

    for i in range(A):
        c_s, r_s, w_s, t_s, hi_s, rank = per_set[i]
        core = t_s // TPC
        tloc = t_s % TPC
        kk = rank // P
        jj = rank % P
        lo_m = hi_s == 0
        col_idx = np.where(lo_m, tloc * cfg.K_u + kk, tloc * cfg.K_u + K_lo + kk)
        colloc[core, i, jj, col_idx] = c_s.astype(np.float32)
        wchunk[core, i, jj, col_idx] = w_s.astype(np.float32)
        gi = idx_in_half[r_s].astype(np.int16)
        pos = tloc * (np.where(lo_m, K_lo, K_hi) * P) + rank
        gidx_lo[core[lo_m], i, pos[lo_m] % 16, pos[lo_m] // 16] = gi[lo_m]
        if cfg.HI_T > 0:
            hi_sel = ~lo_m
            gidx_hi[core[hi_sel], i, pos[hi_sel] % 16, pos[hi_sel] // 16] = gi[hi_sel]

    # --- dense-phase inputs ---
    xpad = np.zeros((cfg.N2, P), np.float32)
    xpad[:N] = x
    x_f16 = xpad.astype(np.float16)                                  # [N2, P]
    x_lo = np.ascontiguousarray(x_f16[lo_nodes])
    x_hi = np.ascontiguousarray(x_f16[hi_nodes]) if cfg.HI_T > 0 else None
    xT_all = np.ascontiguousarray(x_f16.T)                           # [P, N2]
    # WT2[i] = conv_w[i].T : [fin, fout], rhs of the post-aggregation matmul
    WT2 = np.ascontiguousarray(conv_w.transpose(0, 2, 1)).astype(np.float16)
    linWT = np.ascontiguousarray(lin_w.T).astype(np.float16)        # [P,P]
    linb_col = lin_b.reshape(P, 1).astype(np.float32)
    iota_row = np.tile(np.arange(P, dtype=np.float16), (P, 1))       # [P,P]
    b_cols = conv_b.reshape(A, P, 1).astype(np.float32)

    in_maps = []
    for k in range(CORES):
        m = dict(
            x_lo=x_lo,
            xT_own=np.ascontiguousarray(
                xT_all[:, cfg.node_of_slot[k * cfg.OWN:(k + 1) * cfg.OWN]]),
            WT2=WT2, linWT=linWT, linb_col=linb_col,
            iota_row=iota_row, b_cols=b_cols,
            colloc=colloc[k], wchunk=wchunk[k],
            gidx_lo=np.tile(gidx_lo[k], (1, 8, 1)),
        )
        if cfg.HI_T > 0:
            m["x_hi"] = x_hi
            m["gidx_hi"] = np.tile(gidx_hi[k], (1, 8, 1))
        in_maps.append(m)
    return in_maps


def _build(cfg):
    nc = bacc.Bacc(num_swdge_queues=4)
    A, TPC, NT = cfg.A, cfg.TPC, cfg.NT
    K_lo, K_hi, K_u = cfg.K_lo, cfg.K_hi, cfg.K_u
    CH = TPC * K_u
    Alu = mybir.AluOpType
    Act = mybir.ActivationFunctionType

    # ---- I/O ----
    x_lo_d = nc.dram_tensor("x_lo", [cfg.LO_ROWS, P], f16, kind="ExternalInput")
    x_hi_d = (nc.dram_tensor("x_hi", [cfg.HI_ROWS, P], f16, kind="ExternalInput")
              if cfg.HI_T > 0 else None)
    xT_own = nc.dram_tensor("xT_own", [P, cfg.OWN], f16, kind="ExternalInput")
    WT2 = nc.dram_tensor("WT2", [A, P, P], f16, kind="ExternalInput")
    linWT = nc.dram_tensor("linWT", [P, P], f16, kind="ExternalInput")
    linb_col = nc.dram_tensor("linb_col", [P, 1], f32, kind="ExternalInput")
    iota_row = nc.dram_tensor("iota_row", [P, P], f16, kind="ExternalInput")
    b_cols = nc.dram_tensor("b_cols", [A, P, 1], f32, kind="ExternalInput")
    colloc_d = nc.dram_tensor("colloc", [A, P, CH], f32, kind="ExternalInput")
    wchunk_d = nc.dram_tensor("wchunk", [A, P, CH], f32, kind="ExternalInput")
    gidx_lo_d = nc.dram_tensor("gidx_lo", [A, 128, TPC * K_lo * 8], i16,
                               kind="ExternalInput")
    gidx_hi_d = (nc.dram_tensor("gidx_hi", [A, 128, TPC * K_hi * 8], i16,
                                kind="ExternalInput") if cfg.HI_T > 0 else None)

    hs0 = nc.dram_tensor("hs0", [P, cfg.OWN], f16, kind="ExternalOutput")
    outs = [nc.dram_tensor(f"out{i}", [P, cfg.OWN], f16, kind="ExternalOutput")
            for i in range(A)]

    dg = min(DG, TPC)
    own_groups = [(g0, min(dg, TPC - g0)) for g0 in range(0, TPC, dg)]

    with tile.TileContext(nc) as tc:
        with (
            tc.tile_pool(name="const", bufs=1) as cpool,
            tc.tile_pool(name="meta", bufs=1) as mpool,
            tc.tile_pool(name="xw", bufs=2) as xpool,
            tc.tile_pool(name="tts", bufs=6) as tpool,
            tc.tile_pool(name="glo", bufs=4) as glo_pool,
            tc.tile_pool(name="ghi", bufs=4) as ghi_pool,
            tc.tile_pool(name="gix", bufs=3) as gix_pool,
            tc.tile_pool(name="bw", bufs=128) as bwpool,
            tc.tile_pool(name="outst", bufs=3) as opool,
            tc.tile_pool(name="pst", bufs=4, space="PSUM") as pst,
            tc.tile_pool(name="pss", bufs=2, space="PSUM") as pss,
        ):
            # ---- constants to SBUF ----
            # Critical-path constants (first bw builds + first gather) go
            # FIRST on the SP queue; the rest load on the idle Act queue.
            iota_t = cpool.tile([P, P], f16)
            nc.sync.dma_start(out=iota_t[:], in_=iota_row[:])
            # prefetch the first group's gather indices before the big
            # colloc/wchunk loads so gather 0 dispatches immediately
            pre_gix = gix_pool.tile([128, dg * K_lo * 8], i16, tag="gixlo")
            L0 = min(dg, TPC) * K_lo * P
            nc.sync.dma_start(out=pre_gix[:, :L0 // 16],
                              in_=gidx_lo_d[0, :, :L0 // 16])
            pre_gixh = None
            if cfg.HI_T > 0:
                pre_gixh = gix_pool.tile([128, dg * K_hi * 8], i16, tag="gixhi")
                Lh0 = min(dg, TPC) * K_hi * P
                nc.sync.dma_start(out=pre_gixh[:, :Lh0 // 16],
                                  in_=gidx_hi_d[0, :, :Lh0 // 16])
            colloc_t = []
            wchunk_t = []
            for i in range(A):
                ct = mpool.tile([P, CH], f32, tag=f"colloc{i}")
                (nc.sync if i == 0 else nc.scalar).dma_start(
                    out=ct[:], in_=colloc_d[i, :, :])
                colloc_t.append(ct)
                wt = mpool.tile([P, CH], f32, tag=f"wchunk{i}")
                (nc.sync if i == 0 else nc.scalar).dma_start(
                    out=wt[:], in_=wchunk_d[i, :, :])
                wchunk_t.append(wt)
            linb_t = cpool.tile([P, 1], f32)
            nc.scalar.dma_start(out=linb_t[:], in_=linb_col[:])
            linWT_t = cpool.tile([P, P], f16)
            nc.scalar.dma_start(out=linWT_t[:], in_=linWT[:])
            WT2_t = []
            b_t = []
            for i in range(A):
                wt = cpool.tile([P, P], f16, tag=f"WT2{i}")
                nc.scalar.dma_start(out=wt[:], in_=WT2[i, :, :])
                WT2_t.append(wt)
                bt = cpool.tile([P, 1], f32, tag=f"bt{i}")
                nc.scalar.dma_start(out=bt[:], in_=b_cols[i, :, :])
                b_t.append(bt)

            # ---- scatter phase per set, software-pipelined ----
            # The W-matmul + relu of tile j are deferred until after tile
            # j+1's scatter matmuls, so the PE never stalls on the Act cast.
            pending = []   # (set, tts_tile, ot_tile, tl, dma_args)

            def flush_one():
                si, tts_p, ot_p, tl_p, dma_args = pending.pop(0)
                ps2 = pss.tile([P, P], f32, tag="ps2")
                nc.tensor.matmul(out=ps2[:], lhsT=WT2_t[si][:], rhs=tts_p[:],
                                 start=True, stop=True)
                nc.scalar.activation(out=ot_p[:, tl_p * P:(tl_p + 1) * P],
                                     in_=ps2[:], func=Act.Relu,
                                     bias=b_t[si][:, 0:1])
                if dma_args is not None:
                    out_slice, ot_full, tg_p = dma_args
                    nc.scalar.dma_start(out=out_slice, in_=ot_full[:, :tg_p * P])

            # per-set (t0, tg) group lists; the LAST set tapers its final
            # groups (…,4,2,1) so the post-gather pipeline drain is short.
            std_groups = [(g * dg, cfg.group_tiles[g])
                          for g in range(cfg.n_groups)]
            taper_groups = list(std_groups)
            if taper_groups and taper_groups[-1][1] > 1:
                t_last, tg_last = taper_groups.pop()
                splits = []
                rem = tg_last
                while rem > 1:
                    h = (rem + 1) // 2
                    splits.append(h)
                    rem -= h
                splits.append(1)
                for h in splits:
                    taper_groups.append((t_last, h))
                    t_last += h
            for i in range(A):
                glist = taper_groups if i == A - 1 else std_groups
                for t0, tg in glist:
                    L = tg * K_lo * P
                    if i == 0 and t0 == 0:
                        gixt = pre_gix
                    else:
                        gixt = gix_pool.tile([128, dg * K_lo * 8], i16,
                                             tag="gixlo")
                        off = t0 * K_lo * 8
                        nc.sync.dma_start(out=gixt[:, :L // 16],
                                          in_=gidx_lo_d[i, :, off:off + L // 16])
                    glo = glo_pool.tile([P, dg * K_lo, P], f16, tag="glo")
                    nc.gpsimd.dma_gather(
                        out_ap=glo[:, :tg * K_lo, :], in_ap=x_lo_d[:, :],
                        idxs_ap=gixt[:, :L // 16],
                        num_idxs=L, num_idxs_reg=L, elem_size=P,
                        single_packet=False, queue_num=0)
                    if cfg.HI_T > 0:
                        Lh = tg * K_hi * P
                        if i == 0 and t0 == 0:
                            gixh = pre_gixh
                        else:
                            gixh = gix_pool.tile([128, dg * K_hi * 8], i16,
                                                 tag="gixhi")
                            offh = t0 * K_hi * 8
                            nc.sync.dma_start(
                                out=gixh[:, :Lh // 16],
                                in_=gidx_hi_d[i, :, offh:offh + Lh // 16])
                        ghi = ghi_pool.tile([P, dg * K_hi, P], f16, tag="ghi")
                        nc.gpsimd.dma_gather(
                            out_ap=ghi[:, :tg * K_hi, :], in_ap=x_hi_d[:, :],
                            idxs_ap=gixh[:, :Lh // 16],
                            num_idxs=Lh, num_idxs_reg=Lh, elem_size=P,
                            single_packet=False, queue_num=1)
                    ot = opool.tile([P, dg * P], f16, tag="ot")
                    for tl in range(tg):
                        tt = t0 + tl
                        # t^T[fin, c] accumulation
                        psT = pst.tile([P, P], f32, tag="psT")
                        for k in range(K_u):
                            gc = tt * K_u + k
                            # single writer engine per buffer tag
                            # (avoids cross-engine WAW event semaphores)
                            bw = bwpool.tile([P, P], f16, tag="bwv")
                            nc.vector.tensor_scalar(
                                out=bw[:], in0=iota_t[:],
                                scalar1=colloc_t[i][:, gc:gc + 1],
                                scalar2=wchunk_t[i][:, gc:gc + 1],
                                op0=Alu.is_equal, op1=Alu.mult)
                            if k < K_lo:
                                src = glo[:, tl * K_lo + k, :]
                            else:
                                src = ghi[:, tl * K_hi + (k - K_lo), :]
                            nc.tensor.matmul(out=psT[:], lhsT=src, rhs=bw[:],
                                             start=(k == 0), stop=(k == K_u - 1))
                        # cast t^T to fp16 (Act)
                        tts = tpool.tile([P, P], f16, tag="ttsa")
                        nc.scalar.activation(out=tts[:], in_=psT[:],
                                             func=Act.Copy)
                        dma_args = None
                        if tl == tg - 1:
                            g0 = t0 * P
                            dma_args = (outs[i][:, g0:g0 + tg * P], ot, tg)
                        pending.append((i, tts, ot, tl, dma_args))
                        if len(pending) > 1:
                            flush_one()
            while pending:
                flush_one()
            # ---- hs0 (own rows) ----
            for g0, gn in own_groups:
                xo = xpool.tile([P, dg * P], f16, tag="xo")
                nc.sync.dma_start(out=xo[:, :gn * P],
                                  in_=xT_own[:, g0 * P:(g0 + gn) * P])
                obt = opool.tile([P, dg * P], f16, tag="obt")
                for t7 in range(gn):
                    ps = pss.tile([P, P], f32, tag="pso")
                    nc.tensor.matmul(out=ps[:], lhsT=linWT_t[:],
                                     rhs=xo[:, t7 * P:(t7 + 1) * P],
                                     start=True, stop=True)
                    nc.scalar.activation(out=obt[:, t7 * P:(t7 + 1) * P],
                                         in_=ps[:], func=Act.Relu,
                                         bias=linb_t[:, 0:1])
                nc.scalar.dma_start(out=hs0[:, g0 * P:(g0 + gn) * P],
                                    in_=obt[:, :gn * P])

    nc.finalize()
    return nc


def _assemble(cfg, results):
    N, A = cfg.N, cfg.A
    hs = []
    for name in ["hs0"] + [f"out{i}" for i in range(A)]:
        rows = np.concatenate([results[k][name].T for k in range(CORES)],
                              axis=0)
        full = np.empty((cfg.N2, P), np.float32)
        full[cfg.node_of_slot] = rows.astype(np.float32)
        hs.append(full[:N])
    return tuple(hs)


def kernel(x, edge_index, edge_attr, lin_w, lin_b, conv_w, conv_b):
    global LAST_RESULTS
    x = np.asarray(x, np.float32)
    edge_index = np.asarray(edge_index)
    edge_attr = np.asarray(edge_attr, np.float32)
    lin_w = np.asarray(lin_w, np.float32)
    lin_b = np.asarray(lin_b, np.float32)
    conv_w = np.asarray(conv_w, np.float32)
    conv_b = np.asarray(conv_b, np.float32)

    N, D = x.shape
    A, _, E = edge_index.shape
    assert D == P
    cfg = _make_cfg(N, E, A)
    in_maps = _prep(cfg, x, edge_index, edge_attr, lin_w, lin_b, conv_w, conv_b)
    nc = _build(cfg)

    from concourse.bass_utils import run_bass_kernel_spmd
    res = run_bass_kernel_spmd(nc, in_maps, list(range(CORES)), trace=TRACE)
    LAST_RESULTS = res
    return _assemble(cfg, res.results)


# ---------- simulation path (for testing on small configs) ----------

def run_sim(x, edge_index, edge_attr, lin_w, lin_b, conv_w, conv_b,
            cores=None):
    from concourse import bass_interp
    x = np.asarray(x, np.float32)
    edge_index = np.asarray(edge_index)
    edge_attr = np.asarray(edge_attr, np.float32)
    N, D = x.shape
    A, _, E = edge_index.shape
    cfg = _make_cfg(N, E, A)
    in_maps = _prep(cfg, x, edge_index, edge_attr,
                    np.asarray(lin_w, np.float32), np.asarray(lin_b, np.float32),
                    np.asarray(conv_w, np.float32), np.asarray(conv_b, np.float32))
    results = []
    for k in (range(CORES) if cores is None else cores):
        nc = _build(cfg)
        sim = bass_interp.CoreSim(nc, core_id=0, publish_trace=False)
        sim.assign_tensors(in_maps[k])
        sim.simulate()
        results.append({name: sim.tensor(name).copy()
                        for name in ["hs0"] + [f"out{i}" for i in range(A)]})
    if cores is not None:
        return cfg, results
    return _assemble(cfg, results)
